# revision 4
# baseline (speedup 1.0000x reference)
"""Trainium2 Bass kernel for nn_MultiHeadAttention_65352222376626 (v4: + startup/teardown DMA overlap).

v2 algebraic structure (see kernel.py docstring: host-fused M8 = 8*Wq@Wk^T,
A = Wv@Wp, exp-factorized bq bias, appended-denominator output matmul) plus
fp32r acceleration:

  - TRN2 fp32r matmuls run at bf16 speed for moving>=256 and round BOTH
    operands to 12-bit-RNE mantissa (verified on HW: a host Veltkamp q12
    split reproduces the rounding bit-exactly).
  - tmat = x@M8: pass1 fp32r (full fp32 M8 x fp32 x) + pass2 bf16 with
    Mlo = M8 - q12(M8): together exact to ~2^-20.  2 passes instead of 3.
  - tmat hi/lo split on device via the fp16 grid: t16 = fp16(psum) lies
    exactly on the 12-bit fp32r grid, so pass1 stationary upcast(t16) is
    passed through unrounded and pass2's tlo = psum - t16 (bf16) makes the
    t-side exact.
  - scores: pass1 fp32r t*x + pass2 bf16 tlo*xhi.  SCORES_PASSES=3 adds the
    second-order pass3 bf16 bf16(t)*(x - q12(x)) which removes the x-residual
    (sim absmax rel err: 2-pass 0.0133, 3-pass ~0.009; gate is 2e-2).
  - G (bq bias row) single-pass fp32r.
"""

import numpy as np
import ml_dtypes

B, S, D, H = 8, 1024, 768, 12
P = 128
SD = S // P   # 8 tiles along the sequence axis
ED = D // P   # 6 tiles along the feature axis

SCORES_PASSES = 2

_CACHE = {}


def _build_nc(scores_passes=None):
    import concourse.tile as tile
    from concourse import bacc, mybir
    from concourse.masks import make_identity

    if scores_passes is None:
        scores_passes = SCORES_PASSES
    f32 = mybir.dt.float32
    f32r = mybir.dt.float32r
    fp16 = mybir.dt.float16
    bf16 = mybir.dt.bfloat16
    AF = mybir.ActivationFunctionType

    nc = bacc.Bacc()

    # ---- DRAM I/O (per core) ----
    xT_f_d = nc.dram_tensor("xT_f", [D, S], f32r, kind="ExternalInput")
    xT_hi_d = nc.dram_tensor("xT_hi", [D, S], bf16, kind="ExternalInput")
    if scores_passes >= 3:
        xT_lo_d = nc.dram_tensor("xT_lo12", [D, S], bf16, kind="ExternalInput")
    m_f_d = nc.dram_tensor("m_f", [H, D, D], f32r, kind="ExternalInput")
    m_lo_d = nc.dram_tensor("m_lo12", [H, D, D], bf16, kind="ExternalInput")
    a_d = nc.dram_tensor("a_w", [H, D, D], bf16, kind="ExternalInput")
    cT_f_d = nc.dram_tensor("cT_f", [D, H], f32r, kind="ExternalInput")
    out_d = nc.dram_tensor("out", [S, D], f32, kind="ExternalOutput")

    xT_f_t = xT_f_d.rearrange("(o p) s -> p o s", p=P)        # [128, ED, S]
    xT_hi_t = xT_hi_d.rearrange("(o p) s -> p o s", p=P)
    if scores_passes >= 3:
        xT_lo_t = xT_lo_d.rearrange("(o p) s -> p o s", p=P)
    m_f_t = m_f_d.rearrange("h (o p) e -> h p o e", p=P)      # [H, 128, ED, D]
    m_lo_t = m_lo_d.rearrange("h (o p) e -> h p o e", p=P)
    a_t = a_d.rearrange("h (o p) e -> h p o e", p=P)
    cT_f_t = cT_f_d.rearrange("(o p) h -> p o h", p=P)        # [128, ED, H]
    out_t = out_d.rearrange("(o p) d -> p o d", p=P)          # [128, SD, D]

    with tile.TileContext(nc) as tc:
        with (
            tc.tile_pool(name="persist", bufs=1) as persist,
            tc.tile_pool(name="whead", bufs=2) as whead,
            tc.tile_pool(name="qk", bufs=1) as qkpool,
            tc.tile_pool(name="work", bufs=2) as work,
            tc.tile_pool(name="small", bufs=4) as small,
            tc.tile_pool(name="scps", bufs=2, space="PSUM") as scps,
            tc.tile_pool(name="mmps", bufs=2, space="PSUM") as mmps,
            tc.tile_pool(name="prps", bufs=1, space="PSUM") as prps,
        ):
            # ---- persistent tiles ----
            cf = persist.tile([P, ED, H], f32r)
            nc.sync.dma_start(cf[:], cT_f_t)
            xf = persist.tile([P, ED, S], f32r)
            nc.sync.dma_start(xf[:, :, 0:512], xT_f_t[:, :, 0:512])
            nc.sync.dma_start(xf[:, :, 512:1024], xT_f_t[:, :, 512:1024])
            xhi = persist.tile([P, ED, S], bf16)
            nc.sync.dma_start(xhi[:], xT_hi_t)
            if scores_passes >= 3:
                xlo = persist.tile([P, ED, S], bf16)
                nc.sync.dma_start(xlo[:], xT_lo_t)

            ident = persist.tile([P, P], bf16)
            make_identity(nc, ident)
            ident12 = persist.tile([H, H], f32)
            make_identity(nc, ident12)

            acc = persist.tile([P, SD, D], f32)      # final accumulator
            egt = persist.tile([P, SD, H], f32)      # exp(G) transposed
            tf32 = qkpool.tile([P, ED, S], f32r)     # tmat (11-bit values)
            tlo = qkpool.tile([P, ED, S], bf16)      # tmat residual
            if scores_passes >= 3:
                thb = qkpool.tile([P, ED, S], bf16)  # crude bf16 tmat
            va_sb = qkpool.tile([P, SD, D + 1], bf16)

            # ---- G = 8*x@(Wk bq) all heads (1-pass fp32r); EG = exp(G) ----
            eg_sb = work.tile([H, S], f32, tag="scratch")
            g0 = mmps.tile([H, 512], f32, tag="mm")
            g1 = mmps.tile([H, 512], f32, tag="mm")
            for dt in range(ED):
                sh = cf[:, dt, :]
                nc.tensor.matmul(g0[:], sh, xf[:, dt, 0:512],
                                 start=(dt == 0), stop=(dt == ED - 1))
                nc.tensor.matmul(g1[:], sh, xf[:, dt, 512:1024],
                                 start=(dt == 0), stop=(dt == ED - 1))
            nc.scalar.activation(eg_sb[:, 0:512], g0[:], AF.Exp)
            nc.scalar.activation(eg_sb[:, 512:1024], g1[:], AF.Exp)
            for tt in range(SD):
                t_sl = slice(tt * P, (tt + 1) * P)
                egp = mmps.tile([P, H], f32, tag="mm")
                nc.tensor.transpose(egp[:], eg_sb[:, t_sl], ident12[:])
                nc.scalar.copy(egt[:, tt, :], egp[:])

            for h in range(H):
                # ---- per-head weight loads (double-buffered) ----
                mf = whead.tile([P, ED, D], f32r, tag="mf")
                nc.sync.dma_start(mf[:], m_f_t[h])
                mlo = whead.tile([P, ED, D], bf16, tag="mlo")
                nc.sync.dma_start(mlo[:], m_lo_t[h])
                aw = whead.tile([P, ED, D], bf16, tag="aw")
                nc.sync.dma_start(aw[:], a_t[h])

                # ---- tmat = x @ M8_h: fp32r + bf16-residual (2-pass) ----
                for et in range(ED):
                    e_sl = slice(et * P, (et + 1) * P)
                    ps = scps.tile([P, S], f32, tag="sc")
                    for dt in range(ED):
                        sf = mf[:, dt, e_sl]
                        nc.tensor.matmul(ps[:, 0:512], sf, xf[:, dt, 0:512],
                                         start=(dt == 0), stop=False)
                        nc.tensor.matmul(ps[:, 512:1024], sf,
                                         xf[:, dt, 512:1024],
                                         start=(dt == 0), stop=False)
                        sl = mlo[:, dt, e_sl]
                        nc.tensor.matmul(ps[:, 0:512], sl, xhi[:, dt, 0:512],
                                         start=False, stop=(dt == ED - 1))
                        nc.tensor.matmul(ps[:, 512:1024], sl,
                                         xhi[:, dt, 512:1024],
                                         start=False, stop=(dt == ED - 1))
                    # split: t16 on the fp16 grid (exact under fp32r q12);
                    # tlo = psum - t16 in bf16; tf32 = upcast(t16).
                    t16 = work.tile([P, S], fp16, tag="scratch")
                    nc.scalar.activation(t16[:], ps[:], AF.Copy)
                    nc.scalar.activation(tf32[:, et, :], t16[:], AF.Copy)
                    nc.vector.tensor_sub(tlo[:, et, :], ps[:], tf32[:, et, :])
                    if scores_passes >= 3:
                        nc.vector.tensor_copy(thb[:, et, :], t16[:])

                # ---- va = (x @ A_h) * EG, with EG appended as col 768 ----
                for tt in range(SD):
                    t_sl = slice(tt * P, (tt + 1) * P)
                    va0 = mmps.tile([P, 512], f32, tag="mm")
                    va1 = mmps.tile([P, 256], f32, tag="mm")
                    for dt in range(ED):
                        sx = xhi[:, dt, t_sl]
                        nc.tensor.matmul(va0[:], sx, aw[:, dt, 0:512],
                                         start=(dt == 0), stop=(dt == ED - 1))
                        nc.tensor.matmul(va1[:], sx, aw[:, dt, 512:768],
                                         start=(dt == 0), stop=(dt == ED - 1))
                    sc = egt[:, tt, h:h + 1]
                    nc.scalar.mul(va_sb[:, tt, 0:512], va0[:], sc)
                    nc.scalar.mul(va_sb[:, tt, 512:768], va1[:], sc)
                    nc.vector.tensor_copy(va_sb[:, tt, 768:769], sc)

                # ---- scores / softmax / transpose / out-proj, pipelined ----
                def tail(st, ptile):
                    s_sl = slice(st * P, (st + 1) * P)
                    pTs = work.tile([P, SD, P], bf16, tag="pT")
                    for tt in range(SD):
                        t_sl = slice(tt * P, (tt + 1) * P)
                        tpp = mmps.tile([P, 512], bf16, tag="mm")
                        nc.tensor.transpose(tpp[:, 0:P], ptile[:, t_sl],
                                            ident[:])
                        nc.vector.tensor_copy(pTs[:, tt, :], tpp[:, 0:P])
                    pr = prps.tile([P, D + 1], f32, tag="pr")
                    for tt in range(SD):
                        sp = pTs[:, tt, :]
                        nc.tensor.matmul(pr[:, 0:512], sp,
                                         va_sb[:, tt, 0:512],
                                         start=(tt == 0), stop=False)
                        nc.tensor.matmul(pr[:, 512:769], sp,
                                         va_sb[:, tt, 512:769],
                                         start=(tt == 0), stop=(tt == SD - 1))
                    rc = small.tile([P, 1], f32, tag="rc")
                    nc.vector.reciprocal(rc[:], pr[:, D:D + 1])
                    if h == 0:
                        nc.scalar.mul(acc[:, st, :], pr[:, 0:D], rc[:])
                    else:
                        tmp = work.tile([P, D], f32, tag="tmp", bufs=1)
                        nc.scalar.mul(tmp[:], pr[:, 0:D], rc[:])
                        nc.vector.tensor_add(acc[:, st, :], acc[:, st, :],
                                             tmp[:])
                    if h == H - 1:
                        nc.sync.dma_start(out_t[:, st, :], acc[:, st, :])

                prev = None
                for st in range(SD):
                    s_sl = slice(st * P, (st + 1) * P)
                    sc_ps = scps.tile([P, S], f32, tag="sc")
                    for et in range(ED):
                        sf = tf32[:, et, s_sl]
                        nc.tensor.matmul(sc_ps[:, 0:512], sf,
                                         xf[:, et, 0:512],
                                         start=(et == 0), stop=False)
                        nc.tensor.matmul(sc_ps[:, 512:1024], sf,
                                         xf[:, et, 512:1024],
                                         start=(et == 0), stop=False)
                        last = (et == ED - 1) and scores_passes < 3
                        sl = tlo[:, et, s_sl]
                        nc.tensor.matmul(sc_ps[:, 0:512], sl,
                                         xhi[:, et, 0:512],
                                         start=False, stop=last)
                        nc.tensor.matmul(sc_ps[:, 512:1024], sl,
                                         xhi[:, et, 512:1024],
                                         start=False, stop=last)
                        if scores_passes >= 3:
                            sb_ = thb[:, et, s_sl]
                            nc.tensor.matmul(sc_ps[:, 0:512], sb_,
                                             xlo[:, et, 0:512],
                                             start=False, stop=(et == ED - 1))
                            nc.tensor.matmul(sc_ps[:, 512:1024], sb_,
                                             xlo[:, et, 512:1024],
                                             start=False, stop=(et == ED - 1))
                    negmax = small.tile([P, 1], f32, tag="negmax")
                    nc.vector.tensor_reduce(
                        negmax[:], sc_ps[:], axis=mybir.AxisListType.X,
                        op=mybir.AluOpType.max, negate=True)
                    ptile = work.tile([P, S], bf16, tag="p")
                    nc.scalar.activation(ptile[:], sc_ps[:], AF.Exp,
                                         bias=negmax[:])
                    if prev is not None:
                        tail(*prev)
                    prev = (st, ptile)
                tail(*prev)

    nc.compile()
    return nc


def _get_nc():
    if "nc" not in _CACHE:
        _CACHE["nc"] = _build_nc()
    return _CACHE["nc"]


def _q12(a):
    """Round fp32 mantissa to 12 bits RNE (matches TRN2 fp32r operand quant)."""
    a32 = np.asarray(a, np.float32)
    c = np.float32(2 ** 12 + 1)
    s = (a32 * c).astype(np.float32)
    return (s - (s - a32).astype(np.float32)).astype(np.float32)


def _prepare(x, Wq, bq, Wk, bk, Wv, bv, Wp, bp):
    x = np.asarray(x, dtype=np.float32)
    Wq = np.asarray(Wq, dtype=np.float32)
    Wk = np.asarray(Wk, dtype=np.float32)
    Wv = np.asarray(Wv, dtype=np.float32)
    Wp = np.asarray(Wp, dtype=np.float32)
    bq = np.asarray(bq, dtype=np.float32)
    bv = np.asarray(bv, dtype=np.float32)
    bp = np.asarray(bp, dtype=np.float32)

    wp3 = Wp.reshape(H, D, D)
    M8 = 8.0 * np.matmul(Wq, np.transpose(Wk, (0, 2, 1)))
    A = np.matmul(Wv, wp3)
    c8 = 8.0 * np.einsum('hde,he->hd', Wk, bq)
    bp_eff = (bp.astype(np.float64)
              + np.einsum('hd,hde->e', bv.astype(np.float64),
                          wp3.astype(np.float64))).astype(np.float32)

    m_lo = (M8 - _q12(M8)).astype(ml_dtypes.bfloat16)
    a_b = A.astype(ml_dtypes.bfloat16)
    cT_f = np.ascontiguousarray(c8.T)  # [D, H] fp32

    shared = {
        "m_f": M8, "m_lo12": m_lo, "a_w": a_b, "cT_f": cT_f,
    }
    in_maps = []
    for b in range(B):
        xT = np.ascontiguousarray(x[b].T)
        m = {"xT_f": xT, "xT_hi": xT.astype(ml_dtypes.bfloat16), **shared}
        if SCORES_PASSES >= 3:
            m["xT_lo12"] = (xT - _q12(xT)).astype(ml_dtypes.bfloat16)
        in_maps.append(m)
    return in_maps, bp_eff


def kernel(x, Wq, bq, Wk, bk, Wv, bv, Wp, bp):
    from concourse.bass_utils import run_bass_kernel_spmd

    in_maps, bp_eff = _prepare(x, Wq, bq, Wk, bk, Wv, bv, Wp, bp)
    nc = _get_nc()
    res = run_bass_kernel_spmd(nc, in_maps, list(range(B)))
    out = np.stack([res.results[b]["out"] for b in range(B)], axis=0)
    out = out + bp_eff[None, None, :]
    return out.astype(np.float32)
